# revision 40
# baseline (speedup 1.0000x reference)
"""Contrastive loss (SimCLR-style, B=1024, emb [1024,128,128]) on 8 TRN2 cores.

Strategy: host normalizes rows exactly as the reference (dim-1 L2 norm, then
flat-row renorm) and quantizes rn*64 to fp8e4m3; the contraction dim D=16384
(= 128 m x 128 n, m-major) is sharded by m-chunks of 16 across the 8 cores in
DoubleRow layout x[k, n, s, r] = fp8(rn[r, 16c + 2k + s, n] * 64).

sim = rn rn^T is symmetric, so each core computes only the 136 upper-triangle
128x128 tiles (mt-major order), accumulating over its local K=2048 on the PE
in fp8 DoubleRow. Partial tiles are rescaled by 1/32 (so the 8-core sums fit
e4m3's ~240 max), staged tile-major into 2 chunk buffers (48/88 tiles) and
ReduceScattered in fp8: the first chunk's RS rides the fixed ~70us collective
startup window and hides under the remaining gram; the second fires right
after the last matmul. Each core ends up owning 6+11 = 17 fully-summed tiles.
Staging DMAs and psum->sbuf copies alternate between the SP/Activation DMA
queues and DVE/ACT engines to avoid head-of-line stalls.

Post-RS per chunk (pipelined in <=4-tile pieces): exp(sim/T) (ACT), per-tile
row sums (one DVE tensor_reduce over a [p,t,c] view), positives extracted
pre-exp via a host-built diagonal mask (tiles (i,i+8), stt accum), and column
sums as tile^T @ ones matmuls (symmetry: colsum of tile (i,j) feeds row block
j; FWL-fast, overlap the last RS). Output per core is one [128,40] f32; the
host assembles den_r = rowsum_r - e^2 (self-sim is exactly 1) and reduces
loss = (sum log den - 2*sum pos/T) / 2048 in float64.
"""

import numpy as np
import ml_dtypes

import concourse.bacc as bacc
import concourse.mybir as mybir
import concourse.tile as tile
from concourse import bass_utils

F32 = mybir.dt.float32
BF16 = mybir.dt.bfloat16
FP8 = mybir.dt.float8e4
AF = mybir.ActivationFunctionType
ALU = mybir.AluOpType
PM = mybir.MatmulPerfMode

B = 1024
R = 2 * B            # 2048 rows
NCORES = 8
KTILES = 8           # DoubleRow K-tiles per core (256 K each)
NT = 16              # 16x16 grid of 128x128 sim tiles
S = 64.0             # fp8 prescale; sim comes out x S^2
TEMP = 0.5
RSD = 32.0           # post-gram rescale so summed sims fit fp8e4m3 (max 240)
EXPSC = (1.0 / TEMP) * RSD / (S * S)

# upper-triangle tiles in mt-major order
TILES = [(i, j) for i in range(NT) for j in range(i, NT)]      # 136
CHUNKS = [32, 104]                                             # RS chunk sizes
T0 = [0, 32]                                                   # chunk tile base
OWN = [c // NCORES for c in CHUNKS]                            # [4, 13]
STRIP0 = [0, 4]                                                # strip offsets
MAXO = max(OWN)
_NPIECE = [(o + 3) // 4 for o in OWN]        # post-RS processing pieces
_POSC = [sum(_NPIECE[:i]) for i in range(len(OWN))]
_NPOS = sum(_NPIECE)
assert 34 + _NPOS <= 40
NOWN = 17                                                      # tiles per core
STRIPC = NOWN * 128                                            # 2176

_CACHE = {}


NCHUNK = len(CHUNKS)


def _chunk_of(t):
    for ci in range(NCHUNK - 1, -1, -1):
        if t >= T0[ci]:
            return ci
    raise AssertionError


def _build_nc():
    if "nc" in _CACHE:
        return _CACHE["nc"]
    nc = bacc.Bacc("TRN2", target_bir_lowering=False, debug=False,
                   num_devices=NCORES)

    x = nc.dram_tensor("x", [KTILES, 128, 2 * R], FP8, kind="ExternalInput")
    pmask = nc.dram_tensor("pmask", [128, STRIPC], FP8, kind="ExternalInput")
    yrow = nc.dram_tensor("yrow", [128, 40], F32, kind="ExternalOutput")

    cc_in = [nc.dram_tensor(f"cc_in{i}", [CHUNKS[i], 128, 128], FP8)
             for i in range(NCHUNK)]
    cc_rs = [nc.dram_tensor(f"cc_rs{i}", [OWN[i], 128, 128], FP8)
             for i in range(NCHUNK)]
    grp = [list(range(NCORES))]

    with tile.TileContext(nc) as tc:
        with tc.tile_pool(name="x8", bufs=KTILES) as px8, \
             tc.tile_pool(name="simsb", bufs=6) as psb, \
             tc.tile_pool(name="simr", bufs=2) as psimr, \
             tc.tile_pool(name="scr", bufs=2) as pscr, \
             tc.tile_pool(name="pers", bufs=1) as pers, \
             tc.tile_pool(name="ps", bufs=2, space="PSUM") as pps:

            # Exp ACT table preload off the critical path
            junk = pers.tile([128, 16], BF16, tag="junk")
            nc.vector.memset(junk[:], 0.0)
            junk2 = pers.tile([128, 16], F32, tag="junk2")
            nc.scalar.activation(junk2[:], junk[:], AF.Exp)

            # ---- load x (split over both HWDGE queues) ----
            xb = []
            for k in range(KTILES):
                t = px8.tile([128, 2 * R], FP8, tag="x8")
                eng = nc.sync if k % 2 == 0 else nc.scalar
                eng.dma_start(t[0:64, :], x[k, 0:64, :])
                eng.dma_start(t[64:128, :], x[k, 64:128, :])
                xb.append(t)

            expall = pers.tile([128, STRIPC], BF16, tag="expall")
            rowsa = pers.tile([128, 40], F32, tag="rowsa")
            nc.vector.memset(rowsa[:], 0.0)
            ones = pers.tile([128, 1], BF16, tag="ones")
            nc.vector.memset(ones[:], 1.0)

            # ---- gram, upper triangle, fp8 DoubleRow (K=256/inst) ----
            pm = pers.tile([128, STRIPC], FP8, tag="pmask")

            cum = 0
            done_rs = 0
            for mt in range(NT):
                if mt == 1:     # mask load off the hot early DMA window
                    nc.scalar.dma_start(pm[:], pmask[:])
                c0 = 128 * mt
                ps = pps.tile([128, 2048], F32, tag="ps")
                e0 = 512 * (mt // 4) + 512
                col_chunks = [(c0, e0)]
                s = e0
                while s < 2048:
                    col_chunks.append((s, s + 512))
                    s += 512
                for k in range(KTILES):
                    v = xb[k][:].rearrange("p (two n) -> p two n", two=2)
                    lhsT = v[:, :, c0:c0 + 128]
                    for (cs, ce) in col_chunks:
                        nc.tensor.matmul(
                            ps[:, cs:ce], lhsT, v[:, :, cs:ce],
                            start=(k == 0), stop=(k == KTILES - 1),
                            perf_mode=PM.DoubleRow)
                sb = psb.tile([128, 2048], FP8, tag="simsb")
                if mt % 2 == 0:
                    nc.vector.tensor_scalar(
                        sb[:, c0:], ps[:, c0:], 1.0 / RSD, None, ALU.mult)
                else:
                    nc.scalar.activation(
                        sb[:, c0:], ps[:, c0:], AF.Copy, scale=1.0 / RSD)

                # stage tiles into chunk buffers (runs split at boundaries,
                # alternating HWDGE queues, runs capped at 8 tiles)
                n_row = NT - mt
                t = cum
                nrun = 0
                while t < cum + n_row:
                    ci = _chunk_of(t)
                    tb = min(cum + n_row, T0[ci] + CHUNKS[ci], t + 8)
                    l0 = t - T0[ci]
                    d0 = t - cum
                    colv = sb[:, c0 + 128 * d0: c0 + 128 * (d0 + (tb - t))]
                    eng = nc.sync if (mt + nrun) % 2 == 0 else nc.scalar
                    eng.dma_start(
                        cc_in[ci][l0:l0 + (tb - t), :, :].rearrange(
                            "t p c -> p t c"),
                        colv.rearrange("p (t c) -> p t c", c=128))
                    t = tb
                    nrun += 1
                cum += n_row

                # fire RS + per-chunk loss when a chunk completes
                while (done_rs < NCHUNK
                       and cum >= T0[done_rs] + CHUNKS[done_rs]):
                    i = done_rs
                    nc.gpsimd.collective_compute(
                        "ReduceScatter", ALU.add, replica_groups=grp,
                        ins=[cc_in[i][:].opt()], outs=[cc_rs[i][:].opt()])
                    own = OWN[i]
                    s0 = STRIP0[i]
                    simr = psimr.tile([128, MAXO * 128], FP8, tag="simr")
                    scr = pscr.tile([128, MAXO * 128], BF16, tag="scr")
                    # process in <=4-tile pieces: exp/reduce of piece k
                    # overlaps the DMA of piece k+1 (cuts exposed tail)
                    np_ = (own + 3) // 4
                    base = own // np_
                    rem = own - base * np_
                    a = 0
                    for pi in range(np_):
                        b = a + base + (1 if pi < rem else 0)
                        sl = simr[:, 128 * a:128 * b]
                        nc.sync.dma_start(
                            sl.rearrange("p (t c) -> p t c", c=128),
                            cc_rs[i][a:b, :, :].rearrange("t p c -> p t c"))
                        ex = expall[:, 128 * (s0 + a):128 * (s0 + b)]
                        nc.scalar.activation(ex, sl, AF.Exp, scale=EXPSC)
                        nc.vector.tensor_reduce(
                            rowsa[:, s0 + a:s0 + b],
                            ex.rearrange("p (t c) -> p t c", c=128),
                            mybir.AxisListType.X, ALU.add)
                        nc.vector.scalar_tensor_tensor(
                            scr[:, 128 * a:128 * b], sl, EXPSC,
                            pm[:, 128 * (s0 + a):128 * (s0 + b)],
                            ALU.mult, ALU.mult,
                            accum_out=rowsa[:, 34 + _POSC[i] + pi:
                                            35 + _POSC[i] + pi])
                        a = b
                    done_rs += 1

            # ---- column sums: colsum(tile) = tile^T @ ones, [128,1] each ----
            # (FWL-eligible 128-col weight loads; strips of earlier chunks
            # run as soon as the gram frees PSUM, overlapping the last RS)
            psc = pps.tile([128, 2048], F32, tag="ps")
            for tau in range(NOWN):
                nc.tensor.matmul(
                    psc[:, tau:tau + 1],
                    expall[:, 128 * tau:128 * (tau + 1)], ones[:],
                    start=True, stop=True)
            nc.vector.tensor_copy(rowsa[:, 17:17 + NOWN], psc[:, 0:NOWN])

            nc.sync.dma_start(yrow[:], rowsa[:])

    nc.compile()
    _CACHE["nc"] = nc
    return nc


def _owned_tiles(c):
    """(strip_pos, global_tile_idx) pairs owned by core c, strip order."""
    out = []
    for ci in range(NCHUNK):
        for l in range(OWN[ci]):
            out.append((STRIP0[ci] + l, T0[ci] + OWN[ci] * c + l))
    return out


def _make_inputs(emb_i, emb_j):
    ei = np.asarray(emb_i, dtype=np.float32)
    ej = np.asarray(emb_j, dtype=np.float32)
    z = np.concatenate([ei, ej], axis=0)                   # [2048, 128, 128]
    n1 = np.sqrt(np.sum(z * z, axis=1, keepdims=True))
    z = z / np.maximum(n1, 1e-12)
    flat = z.reshape(R, -1)
    fn = np.sqrt(np.sum(flat * flat, axis=1, keepdims=True))
    rn = flat / np.maximum(fn, 1e-8)
    rn8 = (rn * S).astype(ml_dtypes.float8_e4m3).reshape(R, 128, 128)

    in_maps = []
    for c in range(NCORES):
        xc = rn8[:, 16 * c:16 * (c + 1), :]                # [r, 16, n]
        # [r, m, n] -> [k, n, s, r] with m = 2k + s
        xc = xc.transpose(1, 2, 0).reshape(KTILES, 2, 128, R)
        xc = np.ascontiguousarray(xc.transpose(0, 2, 1, 3)).reshape(
            KTILES, 128, 2 * R)
        mask = np.zeros((128, STRIPC), dtype=np.float32)
        for tau, t in _owned_tiles(c):
            i, j = TILES[t]
            if j == i + NCORES:                            # positive-pair tile
                p = np.arange(128)
                mask[p, 128 * tau + p] = 1.0
        in_maps.append({"x": xc,
                        "pmask": mask.astype(ml_dtypes.float8_e4m3)})
    return in_maps


def run(emb_i, emb_j, **spmd_kwargs):
    nc = _build_nc()
    in_maps = _make_inputs(emb_i, emb_j)
    res = bass_utils.run_bass_kernel_spmd(
        nc, in_maps, core_ids=list(range(NCORES)), **spmd_kwargs)

    rows = np.zeros(R, dtype=np.float64)
    pos = 0.0
    for c in range(NCORES):
        yr = np.asarray(res.results[c]["yrow"], dtype=np.float64)
        for tau, t in _owned_tiles(c):
            i, j = TILES[t]
            rows[128 * i:128 * (i + 1)] += yr[:, tau]
            if j != i:
                rows[128 * j:128 * (j + 1)] += yr[:, 17 + tau]
        pos += yr[:, 34:34 + _NPOS].sum()
    den = rows - np.exp(1.0 / TEMP)
    loss = (np.log(den).sum() - 2.0 * pos) / R
    return np.array(loss, dtype=np.float32), res


def kernel(emb_i, emb_j):
    loss, _ = run(emb_i, emb_j)
    return loss
